# revision 1
# baseline (speedup 1.0000x reference)
"""Trainium2 Bass kernel for CausalHolographicQKV — fp8 DoubleRow version.

Math (validated against reference):
  out = IDFT( unit(U_a (.) U_b) (.) (unit(U_w)+unit(U_m)+unit(U_rb)) )
        - 3 * cumsum(x, axis=seq)
  with U_i = half-spectra of the five linear projections, computed as
  x @ (W_i^T F) + fft(b_i) via DFT-basis matmuls.

The -3*cumsum term dominates the output norm (the holographic term is
~6e-4 of it), so cumsum stays fp32 (triangular + prefix matmuls in f32r)
while every dense matmul (W^T F combine, forward transforms, IDFT) runs
in fp8e4 with DoubleRow perf mode (2x128-row contraction per
instruction, 0.5 cycles/row).  unit() makes the holographic term
scale-invariant, so fp8 scaling constants cancel exactly.

Sharding: pure data-parallel over batch (B=8 -> one batch element per
core), cumsum over sequence is fully core-local.  No collectives.
"""

import numpy as np

B, S, D = 8, 2048, 1024
P = 128
NT = S // P          # 16 token tiles per core
ET = D // P          # 8 contraction tiles
NAMES = ["a", "b", "w", "m", "rb"]

# fp8 scale constants (cancel in unit(); cumsum path carries SC explicitly)
S_X = 16.0           # xT8 = S_X * x
S_W = 4096.0         # W8 = S_W * W
S_F = 16.0           # Mb8 = S_F * F (DFT basis)
S_M = 64.0           # M8 = S_M * M  (evac scale = S_M/(S_W*S_F))
S_ONE = 64.0         # onespair value; S_ONE * S_F = S_X * S_M (bias match)
S_C = 16.0           # crt8 = S_C * C
S_B = 8192.0         # Binv8 = S_B * Binv
SC = S_C * S_B       # cumsum terms pre-scaled by SC; final evac /SC

_CACHED = {}


def _f32r(ap):
    import dataclasses
    import concourse.mybir as mybir
    return dataclasses.replace(
        ap, tensor=dataclasses.replace(ap.tensor, dtype=mybir.dt.float32r)
    )


def _make_consts():
    import ml_dtypes
    n = np.arange(D)
    k = np.arange(D // 2)
    ang = 2.0 * np.pi * np.outer(n, k) / D          # [D, 512]
    cos = np.cos(ang)
    msin = -np.sin(ang)
    # forward basis: cols 0..511 Re (cos), cols 512..1023 Im (-sin)
    Mbasis = np.concatenate([cos, msin], axis=1)     # [D, D]
    Mb8 = (S_F * Mbasis).astype(ml_dtypes.float8_e4m3)
    # inverse basis: rows 0..511 weight Re, rows 512..1023 weight Im.
    wk = np.where(k == 0, 1.0, 2.0)
    Cinv = (wk[None, :] * np.cos(ang)).T / D         # [512, D]
    Sinv = (-wk[None, :] * np.sin(ang)).T / D        # [512, D]
    Binv = np.concatenate([Cinv, Sinv], axis=0)
    Binv8 = (S_B * Binv).astype(ml_dtypes.float8_e4m3)
    return Mb8, Binv8


def _build():
    import os
    from contextlib import ExitStack
    nt = int(os.environ.get("KDBG_NT", NT))
    skip_fwd = bool(int(os.environ.get("KDBG_SKIP_FWD", "0")))
    skip_idft = bool(int(os.environ.get("KDBG_SKIP_IDFT", "0")))

    import concourse.bass as bass
    import concourse.tile as tile
    from concourse import bacc, mybir

    f32 = mybir.dt.float32
    f32r = mybir.dt.float32r
    bf16 = mybir.dt.bfloat16
    fp8 = mybir.dt.float8e4
    DR = mybir.MatmulPerfMode.DoubleRow
    AF = mybir.ActivationFunctionType
    OP = mybir.AluOpType

    import ml_dtypes
    Mb8_np, Binv8_np = _make_consts()
    u = np.arange(P)
    ldiag_np = np.where(u[:, None] <= u[None, :], -3.0 * SC, 0.0).astype(np.float32)
    ones_np = np.ones((P, P), dtype=np.float32)
    zeros_np = np.zeros((1, D), dtype=np.float32)
    onespair_np = np.zeros((1, 2, P), dtype=ml_dtypes.float8_e4m3)
    onespair_np[0, 0, :] = S_ONE

    nc = bacc.Bacc("TRN2", target_bir_lowering=False)

    x_d = nc.dram_tensor("x", [S, D], f32r, kind="ExternalInput")
    W_d = {m: nc.dram_tensor(f"W_{m}", [D, D], f32r, kind="ExternalInput")
           for m in NAMES}
    Bf_d = {m: nc.dram_tensor(f"Bf8_{m}", [1, 2, D], fp8, kind="ExternalInput")
            for m in NAMES}
    out_d = nc.dram_tensor("out", [S, D], f32, kind="ExternalOutput")
    Mb_d = nc.inline_tensor(Mb8_np, "Mb8")
    Bi_d = nc.inline_tensor(Binv8_np, "Binv8")
    ldiag_d = nc.inline_tensor(ldiag_np, "ldiag_f32")
    ones_d = nc.inline_tensor(ones_np, "ones_f32")
    onespair_d = nc.inline_tensor(onespair_np, "onespair")
    zeros_d = nc.inline_tensor(zeros_np, "zeros_f32")

    with tile.TileContext(nc) as tc, ExitStack() as ctx:
        const = ctx.enter_context(tc.tile_pool(name="const", bufs=1))
        persist = ctx.enter_context(tc.tile_pool(name="persist", bufs=1))

        ldiag = const.tile([P, P], f32r)
        nc.sync.dma_start(out=ldiag[:, :], in_=_f32r(ldiag_d[:, :]))
        ones_row = const.tile([1, P], f32r)
        nc.sync.dma_start(out=ones_row[:, :], in_=_f32r(ones_d[0:1, :]))
        ones_col = const.tile([P, 1], f32r)
        nc.sync.dma_start(out=ones_col[:, :], in_=_f32r(ones_d[:, 0:1]))
        onespair = const.tile([1, 2, P], fp8)
        nc.sync.dma_start(out=onespair[:, :, :], in_=onespair_d[:, :, :])

        # persistent: M8_i = S_M * W_i^T F (fp8), Bf8_i = S_F * fft(b_i) (fp8)
        M8 = [persist.tile([P, ET, D], fp8, tag=f"M{i}", name=f"M{i}")
              for i in range(5)]
        Bf8 = []
        for i, m in enumerate(NAMES):
            bt = persist.tile([1, 2, D], fp8, tag=f"Bf{i}", name=f"Bf{i}")
            nc.sync.dma_start(out=bt[:, :, :], in_=Bf_d[m][:, :, :])
            Bf8.append(bt)
        P_sb = [persist.tile([1, D], f32r, tag=f"P{i}", name=f"Pst{i}")
                for i in range(3)]
        nc.sync.dma_start(out=P_sb[0][:, :], in_=_f32r(zeros_d[:, :]))

        # ---------- Phase A: M8_i[d, c] = S_M * sum_e W_i[e, d] * F[e, c]
        with tc.tile_pool(name="mb", bufs=1) as mbpool, \
             tc.tile_pool(name="wA", bufs=2) as wpool, \
             tc.tile_pool(name="w8A", bufs=2) as w8pool, \
             tc.tile_pool(name="psA", bufs=2, space="PSUM") as psA:
            Mb = mbpool.tile([P, ET, D], fp8)
            nc.sync.dma_start(
                out=Mb[:, :, :],
                in_=Mb_d[:, :].rearrange("(t p) c -> p t c", p=P))

            for i, m in enumerate(NAMES):
                wt = wpool.tile([P, ET, D], f32r, tag="w", name="wt")
                nc.sync.dma_start(
                    out=wt[:, :, :],
                    in_=W_d[m][:, :].rearrange("(t p) d -> p t d", p=P))
                w8 = w8pool.tile([P, ET, D], fp8, tag="w8", name="w8")
                nc.vector.tensor_scalar_mul(
                    out=w8[:, 0:3, :], in0=wt[:, 0:3, :], scalar1=S_W)
                nc.scalar.mul(w8[:, 3:6, :], wt[:, 3:6, :], S_W)
                nc.gpsimd.tensor_scalar_mul(
                    out=w8[:, 6:8, :], in0=wt[:, 6:8, :], scalar1=S_W)

                for gi in range(16):          # (dm, half) groups
                    dm, half = gi // 2, gi % 2
                    pt = psA.tile([P, 512], f32, tag=f"g{gi % 3}",
                                  name=f"g{gi % 3}")
                    for j in range(4):
                        nc.tensor.matmul(
                            pt[:, :],
                            w8[:, 2 * j:2 * j + 2, dm * P:(dm + 1) * P],
                            Mb[:, 2 * j:2 * j + 2, half * 512:(half + 1) * 512],
                            start=(j == 0), stop=(j == 3),
                            perf_mode=DR,
                        )
                    if gi % 2 == 0:
                        nc.vector.tensor_scalar_mul(
                            out=M8[i][:, dm, half * 512:(half + 1) * 512],
                            in0=pt[:, :], scalar1=S_M / (S_W * S_F))
                    else:
                        nc.scalar.mul(
                            M8[i][:, dm, half * 512:(half + 1) * 512],
                            pt[:, :], S_M / (S_W * S_F))


        # ---------- Phase B: per token tile pipeline
        with tc.tile_pool(name="binv", bufs=1) as bpool, \
             tc.tile_pool(name="xin", bufs=3) as xpool, \
             tc.tile_pool(name="xt", bufs=2) as xtpool, \
             tc.tile_pool(name="ew", bufs=2) as ew, \
             tc.tile_pool(name="crt", bufs=3) as crtpool, \
             tc.tile_pool(name="outp", bufs=2) as outpool, \
             tc.tile_pool(name="psf", bufs=4, space="PSUM") as psf, \
             tc.tile_pool(name="pso", bufs=2, space="PSUM") as pso, \
             tc.tile_pool(name="pss", bufs=1, space="PSUM") as pss:

            Binv = bpool.tile([P, ET, D], fp8)
            nc.sync.dma_start(
                out=Binv[:, :, :],
                in_=Bi_d[:, :].rearrange("(t p) c -> p t c", p=P))

            def act_rsqrt(out, in_):
                eng = nc.scalar
                bias = nc.const_aps.scalar_like(0.0, in_)
                ins = [eng.lower_ap(in_),
                       eng.lower_ap(bias),
                       mybir.ImmediateValue(dtype=mybir.dt.float32, value=1.0),
                       mybir.ImmediateValue(dtype=mybir.dt.float32, value=0.0)]
                return eng.add_instruction(
                    mybir.InstActivation(
                        name=nc.get_next_instruction_name(),
                        func=AF.Rsqrt,
                        ins=ins,
                        outs=[eng.lower_ap(out)],
                    ))

            def vtt(op, a, b, tag, eng=None):
                o = ew.tile([P, 512], bf16, tag=tag, name=tag)
                (eng or nc.vector).tensor_tensor(out=o[:, :], in0=a[:, :],
                                                 in1=b[:, :], op=op)
                return o

            def emit_idft(xt, pcur, crt8, tt):
                """IDFT (fp8 DR) + -3*cumsum (f32r, pre-scaled by SC)."""
                osb = outpool.tile([P, D], f32, tag="out")
                for half in range(2):
                    po = pso.tile([P, 512], f32, tag="od")
                    for j in range(4):
                        nc.tensor.matmul(
                            po[:, :],
                            crt8[:, 2 * j:2 * j + 2, :],
                            Binv[:, 2 * j:2 * j + 2,
                                 half * 512:(half + 1) * 512],
                            start=(j == 0), stop=False,
                            perf_mode=DR,
                        )
                    nc.tensor.matmul(
                        po[:, :],
                        ldiag[:, :],
                        xt[:, half * 512:(half + 1) * 512],
                        start=False, stop=False,
                    )
                    nc.tensor.matmul(
                        po[:, :],
                        ones_row[:, :],
                        pcur[0:1, half * 512:(half + 1) * 512],
                        start=False, stop=True,
                    )
                    nc.scalar.mul(osb[:, half * 512:(half + 1) * 512],
                                  po[:, :], 1.0 / SC)
                nc.sync.dma_start(out=out_d[tt * P:(tt + 1) * P, :],
                                  in_=osb[:, :])

            pend = None
            for tt in range(nt):
                xt = xpool.tile([P, D], f32r, tag="x")
                nc.sync.dma_start(out=xt[:, :],
                                  in_=x_d[tt * P:(tt + 1) * P, :])

                # transpose x tile -> xT (bf16) via DMA xbar, then fp8 scale
                xbf = xtpool.tile([P, D], bf16, tag="xbf", name="xbf")
                nc.gpsimd.tensor_scalar_mul(out=xbf[:, :], in0=xt[:, :],
                                            scalar1=1.0)
                xTt = xtpool.tile([P, ET, P], bf16, tag="xT")
                nc.sync.dma_start_transpose(out=xTt[:, :, :], in_=xbf[:, :])
                xT8 = xtpool.tile([P, ET, P], fp8, tag="xT8", name="xT8")
                nc.gpsimd.tensor_scalar_mul(out=xT8[:, :, :], in0=xTt[:, :, :],
                                            scalar1=S_X)

                # block sum for the running cumsum prefix (pre-scaled by SC)
                pcur = P_sb[tt % 3]
                pnext = P_sb[(tt + 1) % 3]
                for half in range(2):
                    pb = pss.tile([1, 512], f32, tag="bs")
                    nc.tensor.matmul(
                        pb[:, :],
                        ones_col[:, :],
                        xt[:, half * 512:(half + 1) * 512],
                        start=True, stop=True,
                    )
                    nc.vector.scalar_tensor_tensor(
                        out=pnext[0:1, half * 512:(half + 1) * 512],
                        in0=pb[:, :],
                        scalar=-3.0 * SC,
                        in1=pcur[0:1, half * 512:(half + 1) * 512],
                        op0=OP.mult,
                        op1=OP.add,
                    )

                # forward transforms: psum = (S_X x^T)(S_M M_i) + S_ONE*S_F*Bf
                uplanes = []
                for i in range(5):
                    planes = []
                    for half in range(2):
                        pf = psf.tile([P, 512], f32, tag="fw", name="fw")
                        for j in range(4):
                            nc.tensor.matmul(
                                pf[:, :],
                                xT8[:, 2 * j:2 * j + 2, :],
                                M8[i][:, 2 * j:2 * j + 2,
                                      half * 512:(half + 1) * 512],
                                start=(j == 0), stop=False,
                                perf_mode=DR,
                            )
                        nc.tensor.matmul(
                            pf[:, :],
                            onespair[:, :, :],
                            Bf8[i][0:1, :, half * 512:(half + 1) * 512],
                            start=False, stop=True,
                            perf_mode=DR,
                        )
                        planes.append(pf)
                    uplanes.append(planes)

                # previous tile's IDFT fills PE while this tile's element-
                # wise chain produces crt8
                if pend is not None:
                    emit_idft(*pend)
                    pend = None

                # evacuate to bf16 (planes carry S_X*S_M*U -- scale cancels)
                sb = {}
                for i in range(5):
                    for half, sfx in ((0, "r"), (1, "i")):
                        tag = (f"u{i}{sfx}" if i < 2 else f"u{sfx}")
                        t = ew.tile([P, 512], bf16, tag=tag, name=tag)
                        if (i + half) % 2 == 0:
                            nc.vector.tensor_copy(out=t[:, :],
                                                  in_=uplanes[i][half][:, :])
                        else:
                            nc.scalar.copy(out=t[:, :],
                                           in_=uplanes[i][half][:, :])
                        sb[(i, sfx)] = t

                er = {}
                ei = {}
                for i in (2, 3, 4):
                    rr, ri = sb[(i, "r")], sb[(i, "i")]
                    q1 = ew.tile([P, 512], bf16, tag="q1", name="q1")
                    nc.scalar.square(q1[:, :], rr[:, :])
                    q2 = ew.tile([P, 512], bf16, tag="q2", name="q2")
                    nc.scalar.square(q2[:, :], ri[:, :])
                    m2 = vtt(OP.add, q1, q2, "m2")
                    inv = ew.tile([P, 512], bf16, tag="inv", name="inv")
                    act_rsqrt(inv[:, :], m2[:, :])
                    er[i] = vtt(OP.mult, rr, inv, f"er{i}")
                    ei[i] = vtt(OP.mult, ri, inv, f"ei{i}")

                ar, ai = sb[(0, "r")], sb[(0, "i")]
                br, bi = sb[(1, "r")], sb[(1, "i")]
                z1 = vtt(OP.mult, ar, br, "q1")
                z2 = vtt(OP.mult, ai, bi, "q2")
                zr = vtt(OP.subtract, z1, z2, "zr")
                z3 = vtt(OP.mult, ar, bi, "q1")
                z4 = vtt(OP.mult, ai, br, "q2")
                zi = vtt(OP.add, z3, z4, "zi")
                q1 = ew.tile([P, 512], bf16, tag="q1", name="q1z")
                nc.scalar.square(q1[:, :], zr[:, :])
                q2 = ew.tile([P, 512], bf16, tag="q2", name="q2z")
                nc.scalar.square(q2[:, :], zi[:, :])
                mz = vtt(OP.add, q1, q2, "m2")
                izv = ew.tile([P, 512], bf16, tag="izv", name="izv")
                act_rsqrt(izv[:, :], mz[:, :])

                s1 = vtt(OP.add, er[2], er[3], "q1")
                srr = vtt(OP.add, s1, er[4], "srr")
                s2 = vtt(OP.add, ei[2], ei[3], "q2")
                sri = vtt(OP.add, s2, ei[4], "sri")

                p1 = vtt(OP.mult, zr, srr, "q1")
                p2 = vtt(OP.mult, zi, sri, "q2")
                pr = vtt(OP.subtract, p1, p2, "pr")
                p3 = vtt(OP.mult, zr, sri, "q1")
                p4 = vtt(OP.mult, zi, srr, "q2")
                pi = vtt(OP.add, p3, p4, "pi")
                cr = crtpool.tile([P, D], bf16, tag="cr", name="cr")
                nc.vector.tensor_tensor(out=cr[:, 0:512], in0=pr[:, :],
                                        in1=izv[:, :], op=OP.mult)
                nc.vector.tensor_tensor(out=cr[:, 512:1024], in0=pi[:, :],
                                        in1=izv[:, :], op=OP.mult)

                # transpose CR (tok-major -> bin-major) on the Act DMA queue
                crt = crtpool.tile([P, ET, P], bf16, tag="crt")
                nc.scalar.dma_start_transpose(out=crt[:, :, :], in_=cr[:, :])
                crt8 = crtpool.tile([P, ET, P], fp8, tag="crt8", name="crt8")
                nc.gpsimd.tensor_scalar_mul(out=crt8[:, :, :],
                                            in0=crt[:, :, :], scalar1=S_C)
                pend = (xt, pcur, crt8, tt)

            emit_idft(*pend)

    nc.compile()
    return nc


def _get_nc():
    if "nc" not in _CACHED:
        _CACHED["nc"] = _build()
    return _CACHED["nc"]


def kernel(**inputs):
    import ml_dtypes
    from concourse.bass_utils import run_bass_kernel_spmd

    nc = _get_nc()
    x = np.ascontiguousarray(inputs["x"], dtype=np.float32)
    # tiny input prep: pack the 5 bias spectra (5M MACs) as fp8 rows
    bf8 = {}
    for nm in NAMES:
        b = np.asarray(inputs[f"b_{nm}"], dtype=np.float64).reshape(D)
        f = np.fft.rfft(b)[: D // 2]
        row = np.zeros((1, 2, D), dtype=ml_dtypes.float8_e4m3)
        row[0, 0, :] = (S_F * np.concatenate([f.real, f.imag])).astype(
            ml_dtypes.float8_e4m3)
        bf8[nm] = row
    in_maps = []
    for c in range(B):
        m = {"x": x[c]}
        for nm in NAMES:
            m[f"W_{nm}"] = np.ascontiguousarray(inputs[f"W_{nm}"],
                                                dtype=np.float32)
            m[f"Bf8_{nm}"] = bf8[nm]
        in_maps.append(m)
    res = run_bass_kernel_spmd(nc, in_maps, core_ids=list(range(B)))
    out = np.stack([r["out"] for r in res.results], axis=0)
    return out.astype(np.float32)



# revision 61
# speedup vs baseline: 1.2924x; 1.2924x over previous
"""Trainium2 Bass kernel for CausalHolographicQKV — fp8 DoubleRow, v3.

Math (validated against reference):
  out = IDFT( unit(U_a (.) U_b) (.) (unit(U_w)+unit(U_m)+unit(U_rb)) )
        - 3 * cumsum(x, axis=seq)
  with U_i = half-spectra of the five linear projections, computed as
  x @ (W_i^T F) + fft(b_i) via DFT-basis matmuls.

v3: software-pipelined emission.  Each tile's work is split into six
stages (x-prep / forward / magnitude / normalize / combine / IDFT) and
stage k of tile t-k is emitted in loop iteration t, so every engine's
in-order queue only sees instructions whose producers ran >= 1 iteration
earlier — removing the head-of-line semaphore stalls that dominated v2.
W_i ship as fp8 from the host; wide fused DVE ops (stride-0 broadcast
APs) and scalar_tensor_tensor on Pool carry the elementwise chain.

Sharding: pure data-parallel over batch (B=8 -> one batch element per
core), cumsum over sequence is fully core-local.  No collectives.
"""

import dataclasses

import numpy as np

B, S, D = 8, 2048, 1024
P = 128
NT = S // P          # 16 token tiles per core
ET = D // P          # 8 contraction tiles
NAMES = ["a", "b", "w", "m", "rb"]

S_X = 16.0           # xT8 = S_X * x
S_W = 4096.0         # W8 = S_W * W  (host-side cast)
S_F = 16.0           # Mb8 = S_F * F (DFT basis)
S_M = 64.0           # M8 = S_M * M  (evac scale = S_M/(S_W*S_F))
S_ONE = 64.0         # onespair value; S_ONE * S_F = S_X * S_M (bias match)
S_C = 16.0           # crt8 = S_C * C
S_B = 8192.0         # Binv8 = S_B * Binv
SC = S_C * S_B       # cumsum terms pre-scaled by SC; final evac /SC

_CACHED = {}


def _f32r(ap):
    import concourse.mybir as mybir
    return dataclasses.replace(
        ap, tensor=dataclasses.replace(ap.tensor, dtype=mybir.dt.float32r)
    )


def _dims(ap, dims, offset=None):
    """Rewrite an AP's free dims (keeps the partition dim)."""
    import concourse.mybir as mybir
    part = list(ap.ap)[0]
    new = mybir.VecI64Pair([list(part)] + [list(d) for d in dims])
    off = ap.offset if offset is None else offset
    return dataclasses.replace(ap, ap=new, offset=off)


def _make_consts():
    import ml_dtypes
    n = np.arange(D)
    k = np.arange(D // 2)
    ang = 2.0 * np.pi * np.outer(n, k) / D          # [D, 512]
    cos = np.cos(ang)
    msin = -np.sin(ang)
    Mbasis = np.concatenate([cos, msin], axis=1)     # [D, D]
    Mb8 = (S_F * Mbasis).astype(ml_dtypes.float8_e4m3)
    wk = np.where(k == 0, 1.0, 2.0)
    Cinv = (wk[None, :] * np.cos(ang)).T / D         # [512, D]
    Sinv = (-wk[None, :] * np.sin(ang)).T / D        # [512, D]
    Binv = np.concatenate([Cinv, Sinv], axis=0)
    Binv8 = (S_B * Binv).astype(ml_dtypes.float8_e4m3)
    return Mb8, Binv8


def _build():
    import os
    from contextlib import ExitStack
    nt = int(os.environ.get("KDBG_NT", NT))

    import concourse.bass as bass
    import concourse.tile as tile
    from concourse import bacc, mybir

    f32 = mybir.dt.float32
    f32r = mybir.dt.float32r
    bf16 = mybir.dt.bfloat16
    fp8 = mybir.dt.float8e4
    DR = mybir.MatmulPerfMode.DoubleRow
    AF = mybir.ActivationFunctionType
    OP = mybir.AluOpType

    import ml_dtypes
    Mb8_np, Binv8_np = _make_consts()
    u = np.arange(P)
    # -3*SC is a power-of-two multiple of 3 -> exact in bf16
    ldiag_np = np.where(u[:, None] <= u[None, :], -3.0 * SC, 0.0).astype(
        ml_dtypes.bfloat16)
    ones_np = np.ones((P, P), dtype=np.float32)
    zeros_np = np.zeros((1, D), dtype=np.float32)
    onespair_np = np.zeros((1, 2, P), dtype=ml_dtypes.float8_e4m3)
    onespair_np[0, 0, :] = S_ONE

    nc = bacc.Bacc("TRN2", target_bir_lowering=False)

    x_d = nc.dram_tensor("x", [S, D], f32r, kind="ExternalInput")
    W8_d = {m: nc.dram_tensor(f"W8_{m}", [D, D], fp8, kind="ExternalInput")
            for m in NAMES}
    Bf_d = {m: nc.dram_tensor(f"Bf8_{m}", [1, 2, D], fp8, kind="ExternalInput")
            for m in NAMES}
    out_d = nc.dram_tensor("out", [S, D], f32, kind="ExternalOutput")
    Mb_d = nc.inline_tensor(Mb8_np, "Mb8")
    Bi_d = nc.inline_tensor(Binv8_np, "Binv8")
    ldiag_d = nc.inline_tensor(ldiag_np, "ldiag_f32")
    ones_d = nc.inline_tensor(ones_np, "ones_f32")
    onespair_d = nc.inline_tensor(onespair_np, "onespair")
    zeros_d = nc.inline_tensor(zeros_np, "zeros_f32")

    with tile.TileContext(nc) as tc, ExitStack() as ctx:
        const = ctx.enter_context(tc.tile_pool(name="const", bufs=1))
        persist = ctx.enter_context(tc.tile_pool(name="persist", bufs=1))

        # phase-A inputs first: Mb + W8[a] grab the HWDGE/DMA devices before
        # the small const loads so the first matmul can start ~5us in
        mbpool = ctx.enter_context(tc.tile_pool(name="mb", bufs=1))
        wpool = ctx.enter_context(tc.tile_pool(name="wA", bufs=2))
        Mb = mbpool.tile([P, ET, D], fp8)
        nc.sync.dma_start(
            out=Mb[:, :, :],
            in_=Mb_d[:, :].rearrange("(t p) c -> p t c", p=P))
        w8_tiles = {}
        w8_tiles[0] = wpool.tile([P, ET, D], fp8, tag="w8", name="w8")
        nc.sync.dma_start(
            out=w8_tiles[0][:, :, :],
            in_=W8_d[NAMES[0]][:, :].rearrange("(t p) d -> p t d", p=P))

        # small const loads go on the Act HWDGE queue
        ldiag = const.tile([P, P], bf16)
        nc.scalar.dma_start(out=ldiag[:, :], in_=ldiag_d[:, :])
        # full ones matrix: prefix matmul lhs row must share the rhs slot's
        # partition base, so keep a row available at every base
        ones_rows = const.tile([P, P], f32r)
        nc.scalar.dma_start(out=ones_rows[:, :], in_=_f32r(ones_d[:, :]))
        ones_col = const.tile([P, 1], f32r)
        nc.scalar.dma_start(out=ones_col[:, :], in_=_f32r(ones_d[:, 0:1]))
        onespair = const.tile([1, 2, P], fp8)
        nc.scalar.dma_start(out=onespair[:, :, :], in_=onespair_d[:, :, :])

        M8 = [persist.tile([P, ET, D], fp8, tag=f"M{i}", name=f"M{i}")
              for i in range(5)]
        Bf8 = []
        for i, m in enumerate(NAMES):
            bt = persist.tile([1, 2, D], fp8, tag=f"Bf{i}", name=f"Bf{i}")
            nc.scalar.dma_start(out=bt[:, :, :], in_=Bf_d[m][:, :, :])
            Bf8.append(bt)
        # prefix states: one slot per tile boundary, at partition bases
        # {0,32,64} (matmul operands reject base 96) across [128,D] tiles
        NPB = (NT + 1 + 2) // 3
        P_sbs = [persist.tile([P, D], f32r, tag=f"Psb{i}", name=f"Psb{i}")
                 for i in range(NPB)]

        def pslot(i):
            ti, pi = divmod(i, 3)
            return P_sbs[ti][32 * pi:32 * pi + 1, :]

        nc.scalar.dma_start(out=pslot(0), in_=_f32r(zeros_d[:, :]))

        def act_rsqrt(out, in_):
            eng = nc.scalar
            bias = nc.const_aps.scalar_like(0.0, in_)
            ins = [eng.lower_ap(in_),
                   eng.lower_ap(bias),
                   mybir.ImmediateValue(dtype=mybir.dt.float32, value=1.0),
                   mybir.ImmediateValue(dtype=mybir.dt.float32, value=0.0)]
            return eng.add_instruction(
                mybir.InstActivation(
                    name=nc.get_next_instruction_name(),
                    func=AF.Rsqrt,
                    ins=ins,
                    outs=[eng.lower_ap(out)],
                ))

        def pool_tt(out, in0, in1, op):
            # plain tensor_tensor: the only 2-tensor opcode gpsimd supports
            nc.gpsimd.tensor_tensor(out=out, in0=in0, in1=in1, op=op)

        # ---------- Phase A: M8_i[d, c] = S_M * sum_e W_i[e, d] * F[e, c]
        with tc.tile_pool(name="mb", bufs=1) as mbpool, \
             tc.tile_pool(name="wA", bufs=2) as wpool, \
             tc.tile_pool(name="psA", bufs=4, space="PSUM") as psA:
            Mb = mbpool.tile([P, ET, D], fp8)
            nc.sync.dma_start(
                out=Mb[:, :, :],
                in_=Mb_d[:, :].rearrange("(t p) c -> p t c", p=P))

            ecnt = 0
            for i, m in enumerate(NAMES):
                w8 = wpool.tile([P, ET, D], fp8, tag="w8", name="w8")
                nc.sync.dma_start(
                    out=w8[:, :, :],
                    in_=W8_d[m][:, :].rearrange("(t p) d -> p t d", p=P))

                for dm in range(ET):
                    pa = psA.tile([P, 2, 512], f32, tag="pa", name="pa")
                    for j in range(4):
                        for h in range(2):
                            nc.tensor.matmul(
                                pa[:, h, :],
                                w8[:, 2 * j:2 * j + 2, dm * P:(dm + 1) * P],
                                Mb[:, 2 * j:2 * j + 2, h * 512:(h + 1) * 512],
                                start=(j == 0), stop=(j == 3),
                                perf_mode=DR,
                            )
                    sc = S_M / (S_W * S_F)
                    dst = M8[i][:, dm, :]
                    src = pa[:, :, :]
                    # gpsimd cannot read PSUM: alternate DVE/Act
                    if ecnt % 2 == 0:
                        nc.vector.tensor_scalar_mul(out=dst, in0=src, scalar1=sc)
                    else:
                        nc.scalar.mul(dst, src, sc)
                    ecnt += 1

        # ---------- Phase B: 6-stage software pipeline over token tiles
        with tc.tile_pool(name="binv", bufs=1) as bpool, \
             tc.tile_pool(name="ew", bufs=2) as ew, \
             tc.tile_pool(name="psf", bufs=3, space="PSUM") as psf, \
             tc.tile_pool(name="pso", bufs=1, space="PSUM") as pso:

            Binv = bpool.tile([P, ET, D], fp8)
            nc.scalar.dma_start(
                out=Binv[:, :, :],
                in_=Bi_d[:, :].rearrange("(t p) c -> p t c", p=P))

            T = [dict() for _ in range(nt)]

            def st_P(t):
                """x load, bf16 convert, DMA transpose."""
                d = T[t]
                d["xt"] = ew.tile([P, D], f32r, tag="xt", name="xt", bufs=2)
                nc.sync.dma_start(out=d["xt"][:, :],
                                  in_=x_d[t * P:(t + 1) * P, :])
                d["xbf"] = ew.tile([P, D], bf16, tag="xbf", name="xbf", bufs=6)
                nc.gpsimd.tensor_scalar_mul(out=d["xbf"][:, :],
                                            in0=d["xt"][:, :], scalar1=1.0)
                d["xTt"] = ew.tile([P, ET, P], bf16, tag="xTt", name="xTt",
                                   bufs=2)
                nc.sync.dma_start_transpose(out=d["xTt"][:, :, :],
                                            in_=d["xbf"][:, :])

            def st_F(t):
                """fp8 scale, block sum, forward matmuls, pair evacs, prefix."""
                d = T[t]
                d["xT8"] = ew.tile([P, ET, P], fp8, tag="xT8", name="xT8",
                                   bufs=2)
                nc.gpsimd.tensor_scalar_mul(out=d["xT8"][:, :, :],
                                            in0=d["xTt"][:, :, :],
                                            scalar1=S_X)

                pcur = pslot(t)
                pnext = pslot(t + 1)
                d["pcur"] = pcur
                pb = psf.tile([1, 2, 512], f32, tag="fw", name="bs")
                for half in range(2):
                    nc.tensor.matmul(
                        pb[:, half, :],
                        ones_col[:, :],
                        d["xt"][:, half * 512:(half + 1) * 512],
                        start=True, stop=True,
                    )
                # gpsimd cannot read PSUM (pb): DVE hosts the prefix update
                nc.vector.scalar_tensor_tensor(
                    out=pnext[0:1, :],
                    in0=pb[:, :, :],
                    scalar=-3.0 * SC,
                    in1=pcur[0:1, :],
                    op0=OP.mult,
                    op1=OP.add,
                )

                d["UA"] = ew.tile([P, 2, 512], bf16, tag="UA", name="UA",
                                  bufs=2)
                d["UB"] = ew.tile([P, 2, 512], bf16, tag="UB", name="UB",
                                  bufs=2)
                d["UW"] = ew.tile([P, 3, 2, 512], bf16, tag="UW", name="UW",
                                  bufs=3)
                # pair evac engine per proj: all Act (Pool can't read PSUM;
                # DVE is the elementwise bottleneck)
                EV = {0: 1, 1: 1, 2: 1, 3: 1, 4: 1}
                for wave in ((0, 1, 2), (3, 4)):
                    pf = {i: psf.tile([P, 2, 512], f32, tag="fw", name="fw")
                          for i in wave}
                    for j in range(4):
                        for i in wave:
                            for half in range(2):
                                nc.tensor.matmul(
                                    pf[i][:, half, :],
                                    d["xT8"][:, 2 * j:2 * j + 2, :],
                                    M8[i][:, 2 * j:2 * j + 2,
                                          half * 512:(half + 1) * 512],
                                    start=(j == 0), stop=False,
                                    perf_mode=DR,
                                )
                    for i in wave:
                        for half in range(2):
                            nc.tensor.matmul(
                                pf[i][:, half, :],
                                onespair[:, :, :],
                                Bf8[i][0:1, :, half * 512:(half + 1) * 512],
                                start=False, stop=True,
                                perf_mode=DR,
                            )
                    for i in wave:
                        dst = (d["UA"][:, :, :] if i == 0 else
                               d["UB"][:, :, :] if i == 1 else
                               d["UW"][:, i - 2, :, :])
                        src = pf[i][:, :, :]
                        eng = EV[i]
                        if eng == 0:
                            nc.vector.tensor_copy(out=dst, in_=src)
                        elif eng == 1:
                            nc.scalar.copy(out=dst, in_=src)
                        else:
                            nc.gpsimd.tensor_scalar_mul(
                                out=dst, in0=src, scalar1=1.0)

            def st_M(t):
                """DVE magnitude/product block + Pool z-combines."""
                d = T[t]
                ua, ub = d["UA"][:, :, :], d["UB"][:, :, :]
                d["Z13"] = ew.tile([P, 2, 512], bf16, tag="ZP", name="Z13",
                                   bufs=4)
                nc.vector.tensor_tensor(
                    out=d["Z13"][:, :, :],
                    in0=_dims(ua, [[0, 2], [1, 512]], offset=ua.offset),
                    in1=ub, op=OP.mult)
                d["Z24"] = ew.tile([P, 2, 512], bf16, tag="ZP", name="Z24",
                                   bufs=4)
                nc.vector.tensor_tensor(
                    out=d["Z24"][:, :, :],
                    in0=_dims(ua, [[0, 2], [1, 512]], offset=ua.offset + 512),
                    in1=_dims(ub, [[-512, 2], [1, 512]],
                              offset=ub.offset + 512),
                    op=OP.mult)
                d["Q"] = ew.tile([P, 3, 2, 512], bf16, tag="QE", name="Q",
                                 bufs=2)
                nc.vector.tensor_tensor(out=d["Q"][:, :, :, :],
                                        in0=d["UW"][:, :, :, :],
                                        in1=d["UW"][:, :, :, :], op=OP.mult)
                d["M2"] = ew.tile([P, 3, 512], bf16, tag="M2", name="M2",
                                  bufs=2)
                nc.vector.tensor_tensor(out=d["M2"][:, :, :],
                                        in0=d["Q"][:, :, 0, :],
                                        in1=d["Q"][:, :, 1, :], op=OP.add)
                # Pool: z re/im combine
                d["Z"] = ew.tile([P, 2, 512], bf16, tag="Z", name="Z", bufs=3)
                pool_tt(d["Z"][:, 0, :], d["Z13"][:, 0, :], d["Z24"][:, 0, :],
                        OP.subtract)
                pool_tt(d["Z"][:, 1, :], d["Z13"][:, 1, :], d["Z24"][:, 1, :],
                        OP.add)

            def st_NA(t):
                """rsqrt + |z|^2 (Act) and the unit-sum tail (DVE)."""
                d = T[t]
                d["INV"] = ew.tile([P, 3, 512], bf16, tag="INV", name="INV",
                                   bufs=2)
                act_rsqrt(d["INV"][:, :, :], d["M2"][:, :, :])
                d["QZ"] = ew.tile([P, 2, 512], bf16, tag="QS", name="QZ",
                                  bufs=2)
                nc.scalar.square(d["QZ"][:, :, :], d["Z"][:, :, :])

            def st_ND(t):
                """DVE normalize/combine tail."""
                d = T[t]
                d["MZ"] = ew.tile([P, 512], bf16, tag="MZ", name="MZ", bufs=2)
                nc.vector.tensor_tensor(out=d["MZ"][:, :], in0=d["QZ"][:, 0, :],
                                        in1=d["QZ"][:, 1, :], op=OP.add)
                d["E"] = ew.tile([P, 3, 2, 512], bf16, tag="QE", name="E",
                                 bufs=2)
                inv_b = _dims(d["INV"][:, :, :], [[512, 3], [0, 2], [1, 512]])
                nc.vector.tensor_tensor(out=d["E"][:, :, :, :],
                                        in0=d["UW"][:, :, :, :],
                                        in1=inv_b, op=OP.mult)
                d["S1"] = ew.tile([P, 2, 512], bf16, tag="QS", name="S1",
                                  bufs=2)
                nc.vector.tensor_tensor(out=d["S1"][:, :, :],
                                        in0=d["E"][:, 0, :, :],
                                        in1=d["E"][:, 1, :, :], op=OP.add)
                d["SR"] = ew.tile([P, 2, 512], bf16, tag="SR", name="SR",
                                  bufs=2)
                nc.vector.tensor_tensor(out=d["SR"][:, :, :],
                                        in0=d["S1"][:, :, :],
                                        in1=d["E"][:, 2, :, :], op=OP.add)

            def st_B(t):
                """izv, zn, p = zn (.) sr -> CR, transpose."""
                d = T[t]
                d["IZV"] = ew.tile([P, 512], bf16, tag="IZV", name="IZV",
                                   bufs=2)
                act_rsqrt(d["IZV"][:, :], d["MZ"][:, :])
                d["ZN"] = ew.tile([P, 2, 512], bf16, tag="ZN", name="ZN",
                                  bufs=2)
                izv_b = _dims(d["IZV"][:, :], [[0, 2], [1, 512]])
                nc.vector.tensor_tensor(out=d["ZN"][:, :, :],
                                        in0=d["Z"][:, :, :],
                                        in1=izv_b, op=OP.mult)
                d["P12"] = ew.tile([P, 2, 512], bf16, tag="ZP", name="P12",
                                   bufs=4)
                nc.vector.tensor_tensor(out=d["P12"][:, :, :],
                                        in0=d["ZN"][:, :, :],
                                        in1=d["SR"][:, :, :], op=OP.mult)
                d["P34"] = ew.tile([P, 2, 512], bf16, tag="ZP", name="P34",
                                   bufs=4)
                sr_ap = d["SR"][:, :, :]
                nc.vector.tensor_tensor(
                    out=d["P34"][:, :, :],
                    in0=d["ZN"][:, :, :],
                    in1=_dims(sr_ap, [[-512, 2], [1, 512]],
                              offset=sr_ap.offset + 512),
                    op=OP.mult)
                d["CR"] = ew.tile([P, D], bf16, tag="CR", name="CR", bufs=2)
                pool_tt(d["CR"][:, 0:512], d["P12"][:, 0, :],
                        d["P12"][:, 1, :], OP.subtract)
                pool_tt(d["CR"][:, 512:1024], d["P34"][:, 0, :],
                        d["P34"][:, 1, :], OP.add)
                d["crt"] = ew.tile([P, ET, P], bf16, tag="crt", name="crt",
                                   bufs=2)
                nc.sync.dma_start_transpose(out=d["crt"][:, :, :],
                                            in_=d["CR"][:, :])

            def st_I(t):
                """crt8 scale + IDFT + cumsum matmuls."""
                d = T[t]
                d["crt8"] = ew.tile([P, ET, P], fp8, tag="crt8", name="crt8",
                                    bufs=2)
                nc.gpsimd.tensor_scalar_mul(out=d["crt8"][:, :, :],
                                            in0=d["crt"][:, :, :],
                                            scalar1=S_C)
                # drain: the last tiles' IDFTs use the (now idle) psf ring so
                # they don't serialize on the single pso pair
                pool = psf if t >= nt - 5 else pso
                po = pool.tile([P, 2, 512], f32, tag=("fw" if pool is psf
                                                      else "od"), name="od")
                d["po"] = po
                for j in range(4):
                    for h in range(2):
                        nc.tensor.matmul(
                            po[:, h, :],
                            d["crt8"][:, 2 * j:2 * j + 2, :],
                            Binv[:, 2 * j:2 * j + 2, h * 512:(h + 1) * 512],
                            start=(j == 0), stop=False,
                            perf_mode=DR,
                        )
                for h in range(2):
                    nc.tensor.matmul(
                        po[:, h, :],
                        ldiag[:, :],
                        d["xbf"][:, h * 512:(h + 1) * 512],
                        start=False, stop=False,
                    )
                pbase = 32 * (t % 3)
                for h in range(2):
                    nc.tensor.matmul(
                        po[:, h, :],
                        ones_rows[pbase:pbase + 1, :],
                        d["pcur"][0:1, h * 512:(h + 1) * 512],
                        start=False, stop=True,
                    )

            def st_O(t):
                """osb evac + store."""
                d = T[t]
                osb = ew.tile([P, D], f32, tag="osb", name="osb", bufs=2)
                nc.scalar.mul(osb[:, :], d["po"][:, :, :], 1.0 / SC)
                nc.sync.dma_start(out=out_d[t * P:(t + 1) * P, :],
                                  in_=osb[:, :])

            for it in range(nt + 5):
                if 0 <= it - 5 < nt:
                    st_I(it - 5)
                if 0 <= it - 4 < nt:
                    st_B(it - 4)
                if 0 <= it - 3 < nt:
                    st_NA(it - 3)
                if 0 <= it - 2 < nt:
                    st_M(it - 2)
                if 0 <= it - 3 < nt:
                    st_ND(it - 3)
                if 0 <= it - 5 < nt:
                    st_O(it - 5)
                if 0 <= it - 1 < nt:
                    st_F(it - 1)
                if it < nt:
                    st_P(it)

    nc.compile()
    return nc


def _get_nc():
    if "nc" not in _CACHED:
        _CACHED["nc"] = _build()
    return _CACHED["nc"]


def kernel(**inputs):
    import ml_dtypes
    from concourse.bass_utils import run_bass_kernel_spmd

    nc = _get_nc()
    x = np.ascontiguousarray(inputs["x"], dtype=np.float32)
    # input prep: cast weights to fp8 (pre-scaled) and pack the 5 bias
    # spectra (5M MACs) as fp8 rows
    w8 = {}
    bf8 = {}
    for nm in NAMES:
        W = np.asarray(inputs[f"W_{nm}"], dtype=np.float32)
        w8[nm] = (S_W * W).astype(ml_dtypes.float8_e4m3)
        b = np.asarray(inputs[f"b_{nm}"], dtype=np.float64).reshape(D)
        f = np.fft.rfft(b)[: D // 2]
        row = np.zeros((1, 2, D), dtype=ml_dtypes.float8_e4m3)
        row[0, 0, :] = (S_F * np.concatenate([f.real, f.imag])).astype(
            ml_dtypes.float8_e4m3)
        bf8[nm] = row
    in_maps = []
    for c in range(B):
        m = {"x": x[c]}
        for nm in NAMES:
            m[f"W8_{nm}"] = w8[nm]
            m[f"Bf8_{nm}"] = bf8[nm]
        in_maps.append(m)
    res = run_bass_kernel_spmd(nc, in_maps, core_ids=list(range(B)))
    out = np.stack([r["out"] for r in res.results], axis=0)
    return out.astype(np.float32)


# revision 73
# speedup vs baseline: 1.4397x; 1.1140x over previous
"""Trainium2 Bass kernel for CausalHolographicQKV — fp8 DoubleRow, v3.

Math (validated against reference):
  out = IDFT( unit(U_a (.) U_b) (.) (unit(U_w)+unit(U_m)+unit(U_rb)) )
        - 3 * cumsum(x, axis=seq)
  with U_i = half-spectra of the five linear projections, computed as
  x @ (W_i^T F) + fft(b_i) via DFT-basis matmuls.

v3: software-pipelined emission.  Each tile's work is split into six
stages (x-prep / forward / magnitude / normalize / combine / IDFT) and
stage k of tile t-k is emitted in loop iteration t, so every engine's
in-order queue only sees instructions whose producers ran >= 1 iteration
earlier — removing the head-of-line semaphore stalls that dominated v2.
W_i ship as fp8 from the host; wide fused DVE ops (stride-0 broadcast
APs) and scalar_tensor_tensor on Pool carry the elementwise chain.

Sharding: pure data-parallel over batch (B=8 -> one batch element per
core), cumsum over sequence is fully core-local.  No collectives.
"""

import dataclasses

import numpy as np

B, S, D = 8, 2048, 1024
P = 128
NT = S // P          # 16 token tiles per core
ET = D // P          # 8 contraction tiles
NAMES = ["a", "b", "w", "m", "rb"]

S_X = 1.0            # xT8 = cast(x) (fp8 needs no scale)
S_W = 4096.0         # W8 = S_W * W  (host-side cast)
S_F = 16.0           # Mb8 = S_F * F (DFT basis)
S_M = 64.0           # M8 = S_M * M  (evac scale = S_M/(S_W*S_F))
S_ONE = 4.0          # onespair value; S_ONE * S_F = S_X * S_M (bias match)
S_C = 1.0            # crt8 = cast(C)
S_B = 8192.0         # Binv8 = S_B * Binv
SC = S_C * S_B       # cumsum terms pre-scaled by SC; final evac /SC

_CACHED = {}


def _f32r(ap):
    import concourse.mybir as mybir
    return dataclasses.replace(
        ap, tensor=dataclasses.replace(ap.tensor, dtype=mybir.dt.float32r)
    )


def _dims(ap, dims, offset=None):
    """Rewrite an AP's free dims (keeps the partition dim)."""
    import concourse.mybir as mybir
    part = list(ap.ap)[0]
    new = mybir.VecI64Pair([list(part)] + [list(d) for d in dims])
    off = ap.offset if offset is None else offset
    return dataclasses.replace(ap, ap=new, offset=off)


def _make_consts():
    import ml_dtypes
    n = np.arange(D)
    k = np.arange(D // 2)
    ang = 2.0 * np.pi * np.outer(n, k) / D          # [D, 512]
    cos = np.cos(ang)
    msin = -np.sin(ang)
    Mbasis = np.concatenate([cos, msin], axis=1)     # [D, D]
    Mb8 = (S_F * Mbasis).astype(ml_dtypes.float8_e4m3)
    wk = np.where(k == 0, 1.0, 2.0)
    Cinv = (wk[None, :] * np.cos(ang)).T / D         # [512, D]
    Sinv = (-wk[None, :] * np.sin(ang)).T / D        # [512, D]
    Binv = np.concatenate([Cinv, Sinv], axis=0)
    Binv8 = (S_B * Binv).astype(ml_dtypes.float8_e4m3)
    return Mb8, Binv8


def _build():
    import os
    from contextlib import ExitStack
    nt = int(os.environ.get("KDBG_NT", NT))

    import concourse.bass as bass
    import concourse.tile as tile
    from concourse import bacc, mybir

    f32 = mybir.dt.float32
    f32r = mybir.dt.float32r
    bf16 = mybir.dt.bfloat16
    fp8 = mybir.dt.float8e4
    DR = mybir.MatmulPerfMode.DoubleRow
    AF = mybir.ActivationFunctionType
    OP = mybir.AluOpType

    import ml_dtypes
    Mb8_np, Binv8_np = _make_consts()
    u = np.arange(P)
    # -3*SC is a power-of-two multiple of 3 -> exact in bf16
    ldiag_np = np.where(u[:, None] <= u[None, :], -3.0 * SC, 0.0).astype(
        ml_dtypes.bfloat16)
    ones_np = np.ones((P, P), dtype=np.float32)
    zeros_np = np.zeros((1, D), dtype=np.float32)
    onespair_np = np.zeros((1, 2, P), dtype=ml_dtypes.float8_e4m3)
    onespair_np[0, 0, :] = S_ONE

    nc = bacc.Bacc("TRN2", target_bir_lowering=False)

    x_d = nc.dram_tensor("x", [S, D], f32r, kind="ExternalInput")
    W8_d = {m: nc.dram_tensor(f"W8_{m}", [D, D], fp8, kind="ExternalInput")
            for m in NAMES}
    Bf_d = {m: nc.dram_tensor(f"Bf8_{m}", [1, 2, D], fp8, kind="ExternalInput")
            for m in NAMES}
    out_d = nc.dram_tensor("out", [S, D], f32, kind="ExternalOutput")
    Mb_d = nc.inline_tensor(Mb8_np, "Mb8")
    Bi_d = nc.inline_tensor(Binv8_np, "Binv8")
    ldiag_d = nc.inline_tensor(ldiag_np, "ldiag_f32")
    ones_d = nc.inline_tensor(ones_np, "ones_f32")
    onespair_d = nc.inline_tensor(onespair_np, "onespair")
    zeros_d = nc.inline_tensor(zeros_np, "zeros_f32")

    with tile.TileContext(nc) as tc, ExitStack() as ctx:
        const = ctx.enter_context(tc.tile_pool(name="const", bufs=1))
        persist = ctx.enter_context(tc.tile_pool(name="persist", bufs=1))

        # phase-A inputs first: Mb + W8[a] grab the HWDGE/DMA devices before
        # the small const loads so the first matmul can start ~5us in
        phaseA_ctx = ExitStack()
        mbpool = phaseA_ctx.enter_context(tc.tile_pool(name="mb", bufs=1))
        wpool = phaseA_ctx.enter_context(tc.tile_pool(name="wA", bufs=2))
        Mb = mbpool.tile([P, ET, D], fp8)
        nc.sync.dma_start(
            out=Mb[:, :, :],
            in_=Mb_d[:, :].rearrange("(t p) c -> p t c", p=P))
        w8_tiles = {}
        w8_tiles[0] = wpool.tile([P, ET, D], fp8, tag="w8", name="w8")
        nc.sync.dma_start(
            out=w8_tiles[0][:, :, :],
            in_=W8_d[NAMES[0]][:, :].rearrange("(t p) d -> p t d", p=P))

        # small const loads go on the Act HWDGE queue
        ldiag = const.tile([P, P], bf16)
        nc.gpsimd.dma_start(out=ldiag[:, :], in_=ldiag_d[:, :])
        # full ones matrix: prefix matmul lhs row must share the rhs slot's
        # partition base, so keep a row available at every base
        ones_rows = const.tile([P, P], f32r)
        nc.gpsimd.dma_start(out=ones_rows[:, :], in_=_f32r(ones_d[:, :]))
        ones_col = const.tile([P, 1], f32r)
        nc.gpsimd.dma_start(out=ones_col[:, :], in_=_f32r(ones_d[:, 0:1]))
        onespair = const.tile([1, 2, P], fp8)
        nc.gpsimd.dma_start(out=onespair[:, :, :], in_=onespair_d[:, :, :])

        M8 = [persist.tile([P, ET, D], fp8, tag=f"M{i}", name=f"M{i}")
              for i in range(5)]
        Bf8 = []
        for i, m in enumerate(NAMES):
            bt = persist.tile([1, 2, D], fp8, tag=f"Bf{i}", name=f"Bf{i}")
            nc.gpsimd.dma_start(out=bt[:, :, :], in_=Bf_d[m][:, :, :])
            Bf8.append(bt)
        # prefix states: one slot per tile boundary, at partition bases
        # {0,32,64} (matmul operands reject base 96) across [128,D] tiles
        NPB = (NT + 1 + 2) // 3
        P_sbs = [persist.tile([P, D], f32r, tag=f"Psb{i}", name=f"Psb{i}")
                 for i in range(NPB)]

        def pslot(i):
            ti, pi = divmod(i, 3)
            return P_sbs[ti][32 * pi:32 * pi + 1, :]

        nc.gpsimd.dma_start(out=pslot(0), in_=_f32r(zeros_d[:, :]))

        def act_rsqrt(out, in_):
            eng = nc.scalar
            bias = nc.const_aps.scalar_like(0.0, in_)
            ins = [eng.lower_ap(in_),
                   eng.lower_ap(bias),
                   mybir.ImmediateValue(dtype=mybir.dt.float32, value=1.0),
                   mybir.ImmediateValue(dtype=mybir.dt.float32, value=0.0)]
            return eng.add_instruction(
                mybir.InstActivation(
                    name=nc.get_next_instruction_name(),
                    func=AF.Rsqrt,
                    ins=ins,
                    outs=[eng.lower_ap(out)],
                ))

        def pool_tt(out, in0, in1, op):
            # plain tensor_tensor: the only 2-tensor opcode gpsimd supports
            nc.gpsimd.tensor_tensor(out=out, in0=in0, in1=in1, op=op)

        # ---------- Phase A: M8_i[d, c] = S_M * sum_e W_i[e, d] * F[e, c]
        with tc.tile_pool(name="psA", bufs=4, space="PSUM") as psA:
            ecnt = 0
            for i, m in enumerate(NAMES):
                if i in w8_tiles:
                    w8 = w8_tiles[i]
                else:
                    w8 = wpool.tile([P, ET, D], fp8, tag="w8", name="w8")
                    nc.sync.dma_start(
                        out=w8[:, :, :],
                        in_=W8_d[m][:, :].rearrange("(t p) d -> p t d", p=P))

                for dm in range(ET):
                    pa = psA.tile([P, 2, 512], f32, tag="pa", name="pa")
                    for j in range(4):
                        for h in range(2):
                            nc.tensor.matmul(
                                pa[:, h, :],
                                w8[:, 2 * j:2 * j + 2, dm * P:(dm + 1) * P],
                                Mb[:, 2 * j:2 * j + 2, h * 512:(h + 1) * 512],
                                start=(j == 0), stop=(j == 3),
                                perf_mode=DR,
                            )
                    sc = S_M / (S_W * S_F)
                    dst = M8[i][:, dm, :]
                    src = pa[:, :, :]
                    # gpsimd cannot read PSUM: alternate DVE/Act
                    if ecnt % 2 == 0:
                        nc.vector.tensor_scalar_mul(out=dst, in0=src, scalar1=sc)
                    else:
                        nc.scalar.mul(dst, src, sc)
                    ecnt += 1
        phaseA_ctx.close()

        # ---------- Phase B: 6-stage software pipeline over token tiles
        with tc.tile_pool(name="binv", bufs=1) as bpool, \
             tc.tile_pool(name="ew", bufs=2) as ew, \
             tc.tile_pool(name="psf", bufs=3, space="PSUM") as psf, \
             tc.tile_pool(name="pso", bufs=1, space="PSUM") as pso:

            Binv = bpool.tile([P, ET, D], fp8)
            nc.gpsimd.dma_start(
                out=Binv[:, :, :],
                in_=Bi_d[:, :].rearrange("(t p) c -> p t c", p=P))

            T = [dict() for _ in range(nt)]

            def st_P(t):
                """x load, bf16 convert, DMA transpose."""
                d = T[t]
                d["xt"] = ew.tile([P, D], f32r, tag="xt", name="xt", bufs=2)
                nc.sync.dma_start(out=d["xt"][:, :],
                                  in_=x_d[t * P:(t + 1) * P, :])
                d["xbf"] = ew.tile([P, D], bf16, tag="xbf", name="xbf", bufs=6)
                nc.gpsimd.tensor_scalar_mul(out=d["xbf"][:, :],
                                            in0=d["xt"][:, :], scalar1=1.0)
                d["xTt"] = ew.tile([P, ET, P], bf16, tag="xTt", name="xTt",
                                   bufs=2)
                nc.sync.dma_start_transpose(out=d["xTt"][:, :, :],
                                            in_=d["xbf"][:, :])

            def st_Fx(t):
                """xT8 fp8 scale: first in Pool's queue (input 1 iter old)."""
                d = T[t]
                d["xT8"] = ew.tile([P, ET, P], fp8, tag="xT8", name="xT8",
                                   bufs=2)
                nc.gpsimd.tensor_scalar_mul(out=d["xT8"][:, :, :],
                                            in0=d["xTt"][:, :, :],
                                            scalar1=1.0)

            def st_F(t):
                """block sum, forward matmuls, pair evacs, prefix."""
                d = T[t]
                pcur = pslot(t)
                pnext = pslot(t + 1)
                d["pcur"] = pcur
                pb = psf.tile([1, 2, 512], f32, tag="fw", name="bs")
                for half in range(2):
                    nc.tensor.matmul(
                        pb[:, half, :],
                        ones_col[:, :],
                        d["xt"][:, half * 512:(half + 1) * 512],
                        start=True, stop=True,
                    )
                # gpsimd cannot read PSUM (pb): DVE hosts the prefix update
                nc.vector.scalar_tensor_tensor(
                    out=pnext[0:1, :],
                    in0=pb[:, :, :],
                    scalar=-3.0 * SC,
                    in1=pcur[0:1, :],
                    op0=OP.mult,
                    op1=OP.add,
                )


                d["UA"] = ew.tile([P, 2, 512], bf16, tag="UA", name="UA",
                                  bufs=2)
                d["UB"] = ew.tile([P, 2, 512], bf16, tag="UB", name="UB",
                                  bufs=2)
                d["UW"] = ew.tile([P, 3, 2, 512], bf16, tag="UW", name="UW",
                                  bufs=3)
                # pair evac engine per proj: all Act (Pool can't read PSUM;
                # DVE is the elementwise bottleneck)
                EV = {0: 1, 1: 1, 2: 1, 3: 1, 4: 1}
                for wave in ((0, 1, 2), (3, 4)):
                    pf = {i: psf.tile([P, 2, 512], f32, tag="fw", name="fw")
                          for i in wave}
                    for j in range(4):
                        for i in wave:
                            for half in range(2):
                                nc.tensor.matmul(
                                    pf[i][:, half, :],
                                    d["xT8"][:, 2 * j:2 * j + 2, :],
                                    M8[i][:, 2 * j:2 * j + 2,
                                          half * 512:(half + 1) * 512],
                                    start=(j == 0), stop=False,
                                    perf_mode=DR,
                                )
                    for i in wave:
                        for half in range(2):
                            nc.tensor.matmul(
                                pf[i][:, half, :],
                                onespair[:, :, :],
                                Bf8[i][0:1, :, half * 512:(half + 1) * 512],
                                start=False, stop=True,
                                perf_mode=DR,
                            )
                    for i in wave:
                        dst = (d["UA"][:, :, :] if i == 0 else
                               d["UB"][:, :, :] if i == 1 else
                               d["UW"][:, i - 2, :, :])
                        src = pf[i][:, :, :]
                        eng = EV[i]
                        if eng == 0:
                            nc.vector.tensor_copy(out=dst, in_=src)
                        elif eng == 1:
                            nc.scalar.copy(out=dst, in_=src)
                        else:
                            nc.gpsimd.tensor_scalar_mul(
                                out=dst, in0=src, scalar1=1.0)

            def st_M(t):
                """DVE magnitude/product block + Pool z-combines."""
                d = T[t]
                ua, ub = d["UA"][:, :, :], d["UB"][:, :, :]
                d["Z13"] = ew.tile([P, 2, 512], bf16, tag="ZP", name="Z13",
                                   bufs=4)
                nc.vector.tensor_tensor(
                    out=d["Z13"][:, :, :],
                    in0=_dims(ua, [[0, 2], [1, 512]], offset=ua.offset),
                    in1=ub, op=OP.mult)
                d["Z24"] = ew.tile([P, 2, 512], bf16, tag="ZP", name="Z24",
                                   bufs=4)
                nc.vector.tensor_tensor(
                    out=d["Z24"][:, :, :],
                    in0=_dims(ua, [[0, 2], [1, 512]], offset=ua.offset + 512),
                    in1=_dims(ub, [[-512, 2], [1, 512]],
                              offset=ub.offset + 512),
                    op=OP.mult)
                d["Q"] = ew.tile([P, 3, 2, 512], bf16, tag="QE", name="Q",
                                 bufs=2)
                nc.vector.tensor_tensor(out=d["Q"][:, :, :, :],
                                        in0=d["UW"][:, :, :, :],
                                        in1=d["UW"][:, :, :, :], op=OP.mult)
                d["M2"] = ew.tile([P, 4, 512], bf16, tag="M2", name="M2",
                                  bufs=2)
                nc.vector.tensor_tensor(out=d["M2"][:, 0:3, :],
                                        in0=d["Q"][:, :, 0, :],
                                        in1=d["Q"][:, :, 1, :], op=OP.add)
                # Pool: z re/im combine
                d["Z"] = ew.tile([P, 2, 512], bf16, tag="Z", name="Z", bufs=3)
                pool_tt(d["Z"][:, 0, :], d["Z13"][:, 0, :], d["Z24"][:, 0, :],
                        OP.subtract)
                pool_tt(d["Z"][:, 1, :], d["Z13"][:, 1, :], d["Z24"][:, 1, :],
                        OP.add)

            def st_NA(t):
                """|z|^2 (Act square) + MZ into the shared magnitude tile."""
                d = T[t]
                d["QZ"] = ew.tile([P, 2, 512], bf16, tag="QS", name="QZ",
                                  bufs=2)
                nc.scalar.square(d["QZ"][:, :, :], d["Z"][:, :, :])
                nc.vector.tensor_tensor(out=d["M2"][:, 3, :],
                                        in0=d["QZ"][:, 0, :],
                                        in1=d["QZ"][:, 1, :], op=OP.add)

            def st_ND(t):
                """one rsqrt over all four magnitudes + combine tail."""
                d = T[t]
                d["INV"] = ew.tile([P, 4, 512], bf16, tag="INV", name="INV",
                                   bufs=2)
                act_rsqrt(d["INV"][:, :, :], d["M2"][:, :, :])
                d["E"] = ew.tile([P, 3, 2, 512], bf16, tag="QE", name="E",
                                 bufs=2)
                inv_b = _dims(d["INV"][:, 0:3, :], [[512, 3], [0, 2], [1, 512]])
                nc.vector.tensor_tensor(out=d["E"][:, :, :, :],
                                        in0=d["UW"][:, :, :, :],
                                        in1=inv_b, op=OP.mult)
                d["S1"] = ew.tile([P, 2, 512], bf16, tag="QS", name="S1",
                                  bufs=2)
                nc.vector.tensor_tensor(out=d["S1"][:, :, :],
                                        in0=d["E"][:, 0, :, :],
                                        in1=d["E"][:, 1, :, :], op=OP.add)
                d["SR"] = ew.tile([P, 2, 512], bf16, tag="SR", name="SR",
                                  bufs=2)
                nc.vector.tensor_tensor(out=d["SR"][:, :, :],
                                        in0=d["S1"][:, :, :],
                                        in1=d["E"][:, 2, :, :], op=OP.add)

            def st_B(t):
                """izv, zn, p = zn (.) sr -> CR, transpose."""
                d = T[t]
                d["ZN"] = ew.tile([P, 2, 512], bf16, tag="ZN", name="ZN",
                                  bufs=2)
                izv_b = _dims(d["INV"][:, 3, :], [[0, 2], [1, 512]])
                nc.vector.tensor_tensor(out=d["ZN"][:, :, :],
                                        in0=d["Z"][:, :, :],
                                        in1=izv_b, op=OP.mult)
                d["P12"] = ew.tile([P, 2, 512], bf16, tag="ZP", name="P12",
                                   bufs=4)
                nc.vector.tensor_tensor(out=d["P12"][:, :, :],
                                        in0=d["ZN"][:, :, :],
                                        in1=d["SR"][:, :, :], op=OP.mult)
                d["P34"] = ew.tile([P, 2, 512], bf16, tag="ZP", name="P34",
                                   bufs=4)
                sr_ap = d["SR"][:, :, :]
                nc.vector.tensor_tensor(
                    out=d["P34"][:, :, :],
                    in0=d["ZN"][:, :, :],
                    in1=_dims(sr_ap, [[-512, 2], [1, 512]],
                              offset=sr_ap.offset + 512),
                    op=OP.mult)
                d["CR"] = ew.tile([P, D], bf16, tag="CR", name="CR", bufs=2)
                pool_tt(d["CR"][:, 0:512], d["P12"][:, 0, :],
                        d["P12"][:, 1, :], OP.subtract)
                pool_tt(d["CR"][:, 512:1024], d["P34"][:, 0, :],
                        d["P34"][:, 1, :], OP.add)
                d["crt"] = ew.tile([P, ET, P], bf16, tag="crt", name="crt",
                                   bufs=2)
                nc.sync.dma_start_transpose(out=d["crt"][:, :, :],
                                            in_=d["CR"][:, :])

            def st_I(t):
                """crt8 scale + IDFT + cumsum matmuls."""
                d = T[t]
                d["crt8"] = ew.tile([P, ET, P], fp8, tag="crt8", name="crt8",
                                    bufs=2)
                nc.gpsimd.tensor_scalar_mul(out=d["crt8"][:, :, :],
                                            in0=d["crt"][:, :, :],
                                            scalar1=1.0)
                # drain: the last tiles' IDFTs use the (now idle) psf ring so
                # they don't serialize on the single pso pair
                pool = psf if t >= nt - 8 else pso
                po = pool.tile([P, 2, 512], f32, tag=("fw" if pool is psf
                                                      else "od"), name="od")
                d["po"] = po
                for j in range(4):
                    for h in range(2):
                        nc.tensor.matmul(
                            po[:, h, :],
                            d["crt8"][:, 2 * j:2 * j + 2, :],
                            Binv[:, 2 * j:2 * j + 2, h * 512:(h + 1) * 512],
                            start=(j == 0), stop=False,
                            perf_mode=DR,
                        )
                for h in range(2):
                    nc.tensor.matmul(
                        po[:, h, :],
                        ldiag[:, :],
                        d["xbf"][:, h * 512:(h + 1) * 512],
                        start=False, stop=False,
                    )
                pbase = 32 * (t % 3)
                for h in range(2):
                    nc.tensor.matmul(
                        po[:, h, :],
                        ones_rows[pbase:pbase + 1, :],
                        d["pcur"][0:1, h * 512:(h + 1) * 512],
                        start=False, stop=True,
                    )

            def st_O(t):
                """osb evac + store."""
                d = T[t]
                osb = ew.tile([P, D], f32, tag="osb", name="osb", bufs=2)
                nc.scalar.mul(osb[:, :], d["po"][:, :, :], 1.0 / SC)
                nc.sync.dma_start(out=out_d[t * P:(t + 1) * P, :],
                                  in_=osb[:, :])

            for it in range(nt + 5):
                if 0 <= it - 1 < nt:
                    st_Fx(it - 1)
                if 0 <= it - 5 < nt:
                    st_I(it - 5)
                if 0 <= it - 4 < nt:
                    st_B(it - 4)
                if 0 <= it - 3 < nt:
                    st_NA(it - 3)
                if 0 <= it - 2 < nt:
                    st_M(it - 2)
                if 0 <= it - 3 < nt:
                    st_ND(it - 3)
                if 0 <= it - 5 < nt:
                    st_O(it - 5)
                if 0 <= it - 1 < nt:
                    st_F(it - 1)
                if it < nt:
                    st_P(it)

    nc.compile()
    return nc


def _get_nc():
    if "nc" not in _CACHED:
        _CACHED["nc"] = _build()
    return _CACHED["nc"]


def kernel(**inputs):
    import ml_dtypes
    from concourse.bass_utils import run_bass_kernel_spmd

    nc = _get_nc()
    x = np.ascontiguousarray(inputs["x"], dtype=np.float32)
    # input prep: cast weights to fp8 (pre-scaled) and pack the 5 bias
    # spectra (5M MACs) as fp8 rows
    w8 = {}
    bf8 = {}
    for nm in NAMES:
        W = np.asarray(inputs[f"W_{nm}"], dtype=np.float32)
        w8[nm] = (S_W * W).astype(ml_dtypes.float8_e4m3)
        b = np.asarray(inputs[f"b_{nm}"], dtype=np.float64).reshape(D)
        f = np.fft.rfft(b)[: D // 2]
        row = np.zeros((1, 2, D), dtype=ml_dtypes.float8_e4m3)
        row[0, 0, :] = (S_F * np.concatenate([f.real, f.imag])).astype(
            ml_dtypes.float8_e4m3)
        bf8[nm] = row
    in_maps = []
    for c in range(B):
        m = {"x": x[c]}
        for nm in NAMES:
            m[f"W8_{nm}"] = w8[nm]
            m[f"Bf8_{nm}"] = bf8[nm]
        in_maps.append(m)
    res = run_bass_kernel_spmd(nc, in_maps, core_ids=list(range(B)))
    out = np.stack([r["out"] for r in res.results], axis=0)
    return out.astype(np.float32)
